# revision 33
# baseline (speedup 1.0000x reference)
"""Trainium2 Bass kernel for nn_CIP_44392781971895.

Math: the reference computes, per (b, m, t),
    joint[b,m,t] = min( prod_{s,n} pdf(z[b,m,s,n]; mean_T[t,s,n], var[t,s,n])
                        * 4.13273 * std_T0[n], 1e20 )
then num_y = einsum('bmt,tsy', joint, y_true_T), num = sum_t joint,
probs = max(num_y,1e-20)/max(num,1e-20), mean over m, clip to [0,1].

The product over the 512 (s,n) pairs is computed in log space, which
collapses to a matmul over the flattened sn axis:

    logit[t,bm] = CONST + C[t] + sum_sn( A2[sn,t]*z[sn,bm]
                                         + e[sn,t] * (-0.5 z^2)[sn,bm] )
      e  = exp(-log_var_T)   (= 1/var; the reference's 1e-20 variance
           floor binds only for log_var_T < -46, far outside the input
           distribution, so it is not applied)
      A2 = e * mean_T
      C[t] = sum_sn( -0.5*log_var_T - 0.5*e*mean_T^2 )
      CONST = S*N*(log 4.13273 - 0.5 log 2pi) + (S/2) * sum_n log_var_T[0,0,:]
    joint = exp(logit)

The min(.,1e20) clamp is dropped: it binds only for logit > 46, while the
actual log-joints for this input distribution peak at -486 (fp64 check),
hundreds of units below even fp32-exp underflow (-87); bf16 operand
rounding perturbs the logit by a few units at most, so exp underflows to
exactly 0.0 either way and the clamp can never engage.

Sharding: the T=2000 prototype axis is split across the 8 cores (250
each); each core computes a (2, 64, 162) bf16 partial [num_y | num | pad]
over its T-shard, which the host sums (fp32) and finishes.

Structure: tables arrive pre-transposed (sn-major, chunk c = sn c*128+p
on partition p), so stage-1 stationaries (e, A2 and the raw -0.5-fold
tables) come straight from ACT/Pool/DVE elementwise work with no PE
transposes and no PSUM staging copies.  The per-t constants C[t] ride
the same PSUM accumulation as extra matmuls against a -0.5-filled
moving column; CONST rides the exp bias (a tiny f32 column DMA).  The
shard's 250 prototypes split into two PSUM tiles (122 first, 128
second) so each tile's exp / stage-2 matmul / copy / store-DMA overlaps
the other tile's front half.  The A2/qh products are split across Pool
(tile1 + tile0 chunks 0,1) and DVE (tile0 chunks 2,3, after it builds
X), and tile0's PE matmuls issue in chunk-halves gated on whichever
engine's products land first.

Raw Bass (explicit engine blocks + single-event semaphores).  DMA
queues: SP carries zin/cv/mh/ytb, Pool SWDGE carries the two lv tables,
ACT carries nothing so its Exp-table warm starts at t=200; results
stream out through SP (tile1) and ACT (tile0) as soon as each is ready.
"""

from contextlib import ExitStack

import ml_dtypes
import numpy as np

import concourse.bass as bass
import concourse.mybir as mybir

NCORES = 8
B, S, N = 32, 16, 32
T, M, Y = 2000, 2, 10
SN = S * N            # 512  (contraction length)
BM = B * M            # 64   (flattened batch*samples, column index m*B + b)
TSH = T // NCORES     # 250  (prototypes per core)
SY = S * Y            # 160
F32 = mybir.dt.float32
BF16 = mybir.dt.bfloat16
NPBF = ml_dtypes.bfloat16

KONST = float(SN * (np.log(np.float64(4.13273)) - 0.5 * np.log(2.0 * np.pi)))

TP0, TP1 = 128, 122   # tile sizes; tile1 (122 rows) is processed first
OC = SY + 2           # 162 output cols: [num_y(160) | num(1) | pad(1)]


def build_program() -> bass.Bass:
    nc = bass.Bass()
    AF = mybir.ActivationFunctionType
    OP = mybir.AluOpType

    # Packed inputs (built host-side in make_in_maps):
    #   lv0/lv1: [128, 4*tp] bf16  lvT^T chunk-major (chunk c = sn c*128+p)
    #   mh0/mh1: [128, 4*tp] bf16  mT^T same layout
    #   zin:     [128, 768]  bf16  chunk-major [lv dup(64)|mean dup(64)|epsT(64)]
    #   cv:      [128, 1]    f32   CONST (exp bias column)
    #   ytb0/1:  [tp, 162]   bf16  rows t-local: [y(160) | 1 | 0]
    lv0_d = nc.dram_tensor("lv0", [128, 4 * TP0], BF16, kind="ExternalInput")
    lv1_d = nc.dram_tensor("lv1", [128, 4 * TP1], BF16, kind="ExternalInput")
    mh0_d = nc.dram_tensor("mh0", [128, 4 * TP0], BF16, kind="ExternalInput")
    mh1_d = nc.dram_tensor("mh1", [128, 4 * TP1], BF16, kind="ExternalInput")
    zin_d = nc.dram_tensor("zin", [128, 768], BF16, kind="ExternalInput")
    cv_d = nc.dram_tensor("cv", [128, 1], F32, kind="ExternalInput")
    ytb0_d = nc.dram_tensor("ytb0", [TP0, OC], BF16, kind="ExternalInput")
    ytb1_d = nc.dram_tensor("ytb1", [TP1, OC], BF16, kind="ExternalInput")
    part_d = nc.dram_tensor("partial", [2, BM, OC], BF16, kind="ExternalOutput")

    es = ExitStack()
    with es:
        sb = lambda name, shape, dt=BF16: es.enter_context(nc.sbuf_tensor(name, shape, dt))
        ps = lambda name, shape, dt: es.enter_context(nc.psum_tensor(name, shape, dt))

        lv_s = [sb("s_lv0", [128, 4 * TP0]), sb("s_lv1", [128, 4 * TP1])]
        mh_s = [sb("s_mh0", [128, 4 * TP0]), sb("s_mh1", [128, 4 * TP1])]
        e_s = [sb("s_e0", [128, 4 * TP0]), sb("s_e1", [128, 4 * TP1])]
        A2_s = [sb("s_A20", [128, 4 * TP0]), sb("s_A21", [128, 4 * TP1])]
        qh_s = [sb("s_qh0", [128, 4 * TP0]), sb("s_qh1", [128, 4 * TP1])]
        ytb_s = [sb("s_ytb0", [TP0, OC]), sb("s_ytb1", [TP1, OC])]
        joint_s = [sb("s_j0", [TP0, BM]), sb("s_j1", [TP1, BM])]
        zin = sb("s_zin", [128, 768])
        cv_s = sb("s_cv", [128, 1], F32)
        std4 = sb("s_std4", [128, 256])
        X = sb("s_X", [128, 512])            # [zT chunks(4*64) | -0.5 zT^2]
        m05 = sb("s_m05", [128, BM])         # -0.5 fill (fold-matmul moving)
        biasz = sb("s_biasz", [128, 1])      # bf16 zeros (activation bias)
        warm = sb("s_warm", [1, 1])
        ob = [sb("s_ob0", [BM, OC]), sb("s_ob1", [BM, OC])]

        pl = [ps("p_l0", [TP0, BM], F32), ps("p_l1", [TP1, BM], F32)]
        po = [ps("p_o0", [BM, OC], F32), ps("p_o1", [BM, OC], F32)]

        sem = lambda name: es.enter_context(nc.semaphore(name))
        s_zin, s_cv = sem("s_zin"), sem("s_cvs")
        s_lv = [sem("s_lv0s"), sem("s_lv1s")]
        s_mh = [sem("s_mh0s"), sem("s_mh1s")]
        s_ytb = [sem("s_ytb0s"), sem("s_ytb1s")]
        s_const, s_std, s_X = sem("s_const"), sem("s_std"), sem("s_Xs")
        s_e = [sem("s_e0s"), sem("s_e1s")]
        s_p = [sem("s_p0"), sem("s_p1")]     # Pool products done (tile)
        s_d0 = sem("s_d0")                   # DVE products done (tile0 c2c3)
        s_mm = [sem("s_mm0"), sem("s_mm1")]
        s_j = [sem("s_j0s"), sem("s_j1s")]
        s_s2 = [sem("s_s20"), sem("s_s21")]
        s_ob = [sem("s_ob0s"), sem("s_ob1s")]
        s_od = [sem("s_od0"), sem("s_od1")]

        zview = zin[:].rearrange("p (c k) -> p c k", k=192)
        lv4 = zview[:, :, 0:BM]
        mean4 = zview[:, :, BM:2 * BM]
        eps4 = zview[:, :, 2 * BM:3 * BM]
        std4v = std4[:].rearrange("p (c k) -> p c k", k=BM)
        X0v = X[:, 0:256].rearrange("p (c k) -> p c k", k=BM)
        TPS = [TP0, TP1]

        with nc.Block() as block:

            @block.sync
            def _(sync):
                sync.dma_start(zin[:], zin_d[:]).then_inc(s_zin, 16)
                sync.dma_start(cv_s[:], cv_d[:]).then_inc(s_cv, 16)
                sync.dma_start(mh_s[1][:], mh1_d[:]).then_inc(s_mh[1], 16)
                sync.dma_start(mh_s[0][:], mh0_d[:]).then_inc(s_mh[0], 16)
                sync.dma_start(ytb_s[1][:], ytb1_d[:]).then_inc(s_ytb[1], 16)
                sync.dma_start(ytb_s[0][:], ytb0_d[:]).then_inc(s_ytb[0], 16)
                sync.wait_ge(s_ob[1], 1)
                sync.dma_start(part_d[1], ob[1][:]).then_inc(s_od[1], 16)

            @block.scalar
            def _(scalar):
                # warm the ACT Exp table from t=200 while DMAs are in flight
                scalar.wait_ge(s_const, 1)
                scalar.activation(warm[:], biasz[0:1, :], AF.Exp,
                                  bias=biasz[0:1, :])
                scalar.wait_ge(s_zin, 16)
                scalar.activation(std4[:], lv4, AF.Exp, bias=biasz[:, :],
                                  scale=0.5).then_inc(s_std, 1)
                for ti in (1, 0):
                    scalar.wait_ge(s_lv[ti], 16)
                    scalar.activation(e_s[ti][:], lv_s[ti][:], AF.Exp,
                                      bias=biasz[:, :],
                                      scale=-1.0).then_inc(s_e[ti], 1)
                scalar.wait_ge(s_cv, 16)
                for ti in (1, 0):
                    scalar.wait_ge(s_mm[ti], 1)
                    scalar.activation(joint_s[ti][:], pl[ti][:], AF.Exp,
                                      bias=cv_s[:TPS[ti], :]).then_inc(s_j[ti], 1)
                scalar.wait_ge(s_s2[0], 1)
                scalar.copy(ob[0][:], po[0][:]).then_inc(s_ob[0], 1)
                scalar.wait_ge(s_ob[0], 1)
                scalar.dma_start(part_d[0], ob[0][:]).then_inc(s_od[0], 16)

            @block.gpsimd
            def _(gp):
                gp.memset(m05[:], -0.5)
                gp.memset(biasz[:], 0.0).then_inc(s_const, 1)
                gp.dma_start(lv_s[1][:], lv1_d[:]).then_inc(s_lv[1], 16)
                gp.dma_start(lv_s[0][:], lv0_d[:]).then_inc(s_lv[0], 16)
                # tile1 products
                gp.wait_ge(s_e[1], 1)
                gp.wait_ge(s_mh[1], 16)
                gp.tensor_mul(A2_s[1][:], e_s[1][:], mh_s[1][:])
                gp.drain()
                gp.tensor_mul(qh_s[1][:], A2_s[1][:], mh_s[1][:]).then_inc(s_p[1], 1)
                # tile0 chunks 0,1
                gp.wait_ge(s_e[0], 1)
                gp.wait_ge(s_mh[0], 16)
                gp.tensor_mul(A2_s[0][:, 0:2 * TP0], e_s[0][:, 0:2 * TP0],
                              mh_s[0][:, 0:2 * TP0])
                gp.drain()
                gp.tensor_mul(qh_s[0][:, 0:2 * TP0], A2_s[0][:, 0:2 * TP0],
                              mh_s[0][:, 0:2 * TP0]).then_inc(s_p[0], 1)

            @block.vector
            def _(vector):
                vector.wait_ge(s_zin, 16)
                vector.wait_ge(s_std, 1)
                vector.tensor_mul(X0v, eps4, std4v)
                vector.drain()
                vector.tensor_add(X0v, X0v, mean4)
                vector.drain()
                vector.scalar_tensor_tensor(
                    X[:, 256:512], X[:, 0:256], -0.5, X[:, 0:256],
                    op0=OP.mult, op1=OP.mult).then_inc(s_X, 1)
                # tile0 chunks 2,3
                vector.wait_ge(s_e[0], 1)
                vector.wait_ge(s_mh[0], 16)
                vector.tensor_mul(A2_s[0][:, 2 * TP0:4 * TP0], e_s[0][:, 2 * TP0:4 * TP0],
                                  mh_s[0][:, 2 * TP0:4 * TP0])
                vector.drain()
                vector.tensor_mul(qh_s[0][:, 2 * TP0:4 * TP0], A2_s[0][:, 2 * TP0:4 * TP0],
                                  mh_s[0][:, 2 * TP0:4 * TP0]).then_inc(s_d0, 1)
                vector.wait_ge(s_s2[1], 1)
                vector.tensor_copy(ob[1][:], po[1][:]).then_inc(s_ob[1], 1)

            @block.tensor
            def _(tensor):
                # tile1 (122 rows) first: its exp/stage-2/store overlaps tile0.
                def fold_mms(ti, tbl, start):
                    tp = TPS[ti]
                    for c in range(4):
                        ins = nc.tensor.matmul(pl[ti][:], tbl[:, c * tp:(c + 1) * tp],
                                               m05[:], start=(start and c == 0),
                                               stop=False)
                    return ins

                def z_mms(ti):
                    tp = TPS[ti]
                    for c in range(4):
                        nc.tensor.matmul(pl[ti][:], A2_s[ti][:, c * tp:(c + 1) * tp],
                                         X[:, c * BM:(c + 1) * BM],
                                         start=False, stop=False)
                    for c in range(4):
                        ins = nc.tensor.matmul(pl[ti][:], e_s[ti][:, c * tp:(c + 1) * tp],
                                               X[:, 256 + c * BM:256 + (c + 1) * BM],
                                               start=False, stop=(c == 3))
                    return ins

                tensor.wait_ge(s_const, 1)
                tensor.wait_ge(s_lv[1], 16)
                fold_mms(1, lv_s[1][:], start=True)
                tensor.wait_ge(s_lv[0], 16)
                fold_mms(0, lv_s[0][:], start=True)
                # per-half blocks: qh-folds then A2*z then e*(-z^2/2)
                def half_mms(ti, cs, stop_c):
                    tp = TPS[ti]
                    for c in cs:
                        nc.tensor.matmul(pl[ti][:], qh_s[ti][:, c * tp:(c + 1) * tp],
                                         m05[:], start=False, stop=False)
                    for c in cs:
                        nc.tensor.matmul(pl[ti][:], A2_s[ti][:, c * tp:(c + 1) * tp],
                                         X[:, c * BM:(c + 1) * BM],
                                         start=False, stop=False)
                    for c in cs:
                        ins = nc.tensor.matmul(pl[ti][:], e_s[ti][:, c * tp:(c + 1) * tp],
                                               X[:, 256 + c * BM:256 + (c + 1) * BM],
                                               start=False, stop=(c == stop_c))
                    return ins

                tensor.wait_ge(s_p[1], 1)
                tensor.wait_ge(s_X, 1)
                half_mms(1, (0, 1), -1)
                half_mms(1, (2, 3), 3).then_inc(s_mm[1], 1)

                # tile0: DVE's chunks 2,3 finish before Pool's 0,1
                tensor.wait_ge(s_d0, 1)
                half_mms(0, (2, 3), -1)
                tensor.wait_ge(s_p[0], 1)
                half_mms(0, (0, 1), 1).then_inc(s_mm[0], 1)
                for ti in (1, 0):
                    tensor.wait_ge(s_j[ti], 1)
                    tensor.wait_ge(s_ytb[ti], 16)
                    nc.tensor.matmul(po[ti][:], joint_s[ti][:],
                                     ytb_s[ti][:, :],
                                     start=True, stop=True).then_inc(s_s2[ti], 1)

    nc.finalize()
    return nc


_PROG = None


def _get_prog() -> bass.Bass:
    global _PROG
    if _PROG is None:
        _PROG = build_program()
    return _PROG


def make_in_maps(mean, log_var, mean_T, log_var_T, y_true_T, eps):
    f = np.float32
    mean32 = np.asarray(mean, f).reshape(B, SN)
    lv32 = np.asarray(log_var, f).reshape(B, SN)
    eps32 = np.asarray(eps, f).reshape(BM, SN)
    lvT = np.asarray(log_var_T, f).reshape(T, SN)
    mT = np.asarray(mean_T, f).reshape(T, SN)
    yT = np.asarray(y_true_T, f).reshape(T, SY)

    cval = f(KONST + (S * 0.5) * np.sum(lvT[0, :N], dtype=np.float64))
    cv = np.full((128, 1), cval, f)

    # sn-major z inputs, m-duplicated to 64 columns (bm = m*B + b)
    lvd = np.tile(lv32.T, (1, M))                                 # (512, 64)
    mnd = np.tile(mean32.T, (1, M))
    epT = eps32.T                                                 # (512, 64)
    full = np.concatenate([lvd, mnd, epT], axis=1)                # (512, 192)
    zin = np.ascontiguousarray(
        full.reshape(4, 128, 192).transpose(1, 0, 2).reshape(128, 768)
    ).astype(NPBF)

    def packT(tblT, t0, tp):
        # tblT: (512, 250) shard slice -> [128, 4*tp] chunk-major bf16
        return np.ascontiguousarray(np.concatenate(
            [tblT[c * 128:(c + 1) * 128, t0:t0 + tp] for c in range(4)],
            axis=1)).astype(NPBF)

    in_maps = []
    for core in range(NCORES):
        sl = slice(core * TSH, (core + 1) * TSH)
        lvTT = lvT[sl].T                                          # (512, 250)
        mTT = mT[sl].T
        ytb = np.zeros((TSH, OC), f)
        ytb[:, :SY] = yT[sl]
        ytb[:, SY] = 1.0
        in_maps.append({
            "lv0": packT(lvTT, 0, TP0),
            "lv1": packT(lvTT, TP0, TP1),
            "mh0": packT(mTT, 0, TP0),
            "mh1": packT(mTT, TP0, TP1),
            "zin": zin,
            "cv": cv,
            "ytb0": np.ascontiguousarray(ytb[0:TP0]).astype(NPBF),
            "ytb1": np.ascontiguousarray(ytb[TP0:TSH]).astype(NPBF),
        })
    return in_maps


def finish(partials) -> np.ndarray:
    """Host epilogue: sum per-core/per-tile partials, divide, mean, clip."""
    tot = np.sum(np.stack([np.asarray(p, np.float32).reshape(-1, BM, OC)
                           for p in partials]), axis=(0, 1), dtype=np.float32)
    num_y = tot[:, :SY].reshape(M, B, S, Y)
    num_j = tot[:, SY].reshape(M, B, 1, 1)
    probs = np.maximum(num_y, np.float32(1e-20)) / np.maximum(num_j, np.float32(1e-20))
    prob = np.sum(probs, axis=0, dtype=np.float32) / np.float32(M)
    return np.clip(prob, 0.0, 1.0).astype(np.float32)


def kernel(mean, log_var, mean_T, log_var_T, y_true_T, eps) -> np.ndarray:
    from concourse.bass_utils import run_bass_kernel_spmd

    nc = _get_prog()
    in_maps = make_in_maps(mean, log_var, mean_T, log_var_T, y_true_T, eps)
    res = run_bass_kernel_spmd(nc, in_maps, list(range(NCORES))).results
    return finish([r["partial"] for r in res])
